# revision 3
# baseline (speedup 1.0000x reference)
"""Trainium2 Bass kernel for nn_Decoder_21595095564698.

Strategy
--------
Pure data parallel over batch (16384 -> 8 cores x 2048). Key algebraic
simplifications baked in (validated against the reference to 2e-7 in fp32):

* The attention softmax is over a single element -> attn == 1.0, so
  context == V every step; the whole Q/scores path is dead code.
* V and Vg = V @ Wg_c.T (+ biases) are fused into one [61, 60] linear map
  applied to the encoder mean, with a ones-row providing the biases.
* K/V depend only on mean_l(encoder_outputs): the 188 MB encoder tensor is
  reduced on-device with accumulating DMAs (SDMA inline adds), so no compute
  engine touches the bulk data.
* The 12-step recurrence runs in a 4-chunk "banded" layout: batch chunk c
  occupies SBUF partitions [32c, 32c+30); all matmuls are 16 concurrent
  32x32 PE-subarray tiles (tile_position), elementwise work runs on
  [128, 512] tiles at full lane occupancy.
* Matmul operands are fp16 (1 cyc/row on the PE; ~5e-4 rel rounding);
  the LSTM cell state c stays fp32. End-to-end L2 rel err ~7e-4.
"""

import numpy as np

B, T, DIN, ENC_LEN, ENC, DEC = 16384, 13, 4, 48, 60, 30
NCORES = 8
BC = B // NCORES          # 2048 batch per core
NB = 4                    # chunks per core (partition bands)
ROW_MAP = [0, 30, 90, 60]  # LSTM gate banks: i, f, o, g(cell) -> W row offsets

_PROG = None  # cached (nc, core_ids)


def _local_of_dev():
    d = np.arange(BC)
    c, r = d // 512, d % 512
    jp, p = r // 128, r % 128
    return 16 * p + 4 * c + jp


def _build_program():
    import sys
    if '/opt/trn_rl_repo' not in sys.path:
        sys.path.insert(0, '/opt/trn_rl_repo')
    import concourse.bass as bass  # noqa
    import concourse.tile as tile
    from concourse import bacc, mybir
    from concourse.masks import make_identity

    F32 = mybir.dt.float32
    F16 = mybir.dt.float16
    AF = mybir.ActivationFunctionType
    OP = mybir.AluOpType

    nc = bacc.Bacc("TRN2", target_bir_lowering=False, debug=False,
                   num_devices=NCORES)

    # ---- I/O (per-core shapes) ----
    enc_d = nc.dram_tensor("enc", [ENC_LEN, BC, ENC], F32, kind="ExternalInput").ap()
    yt_d = nc.dram_tensor("yt", [52, BC], F16, kind="ExternalInput").ap()
    h0_d = nc.dram_tensor("h0", [128, 512], F16, kind="ExternalInput").ap()
    wkv_d = nc.dram_tensor("wkv61", [61, 60], F16, kind="ExternalInput").ap()
    wgh_d = nc.dram_tensor("wgh", [128, 30], F16, kind="ExternalInput").ap()
    id30_d = nc.dram_tensor("id30", [128, 30], F16, kind="ExternalInput").ap()
    wenh_d = nc.dram_tensor("w_enh", [128, 120], F16, kind="ExternalInput").ap()
    whh_d = nc.dram_tensor("w_hh", [128, 120], F16, kind="ExternalInput").ap()
    wys_d = nc.dram_tensor("wys", [52, 12, 120], F16, kind="ExternalInput").ap()
    wout1_d = nc.dram_tensor("wout1", [1, 120], F16, kind="ExternalInput").ap()
    wf1_d = nc.dram_tensor("wf1", [128, 30], F16, kind="ExternalInput").ap()
    wf2_d = nc.dram_tensor("wf2", [128, 15], F16, kind="ExternalInput").ap()
    wf3_d = nc.dram_tensor("wf3", [128, 1], F16, kind="ExternalInput").ap()
    b4_d = nc.dram_tensor("b4", [128, 4], F32, kind="ExternalInput").ap()
    b1s_d = nc.dram_tensor("b1s", [128, 12], F32, kind="ExternalInput").ap()
    b2_d = nc.dram_tensor("b2", [128, 1], F32, kind="ExternalInput").ap()
    bf3_d = nc.dram_tensor("bf3", [1, 1], F32, kind="ExternalInput").ap()
    out_d = nc.dram_tensor("out", [12, BC], F32, kind="ExternalOutput").ap()

    with tile.TileContext(nc) as tc:
        with tc.tile_pool(name="const", bufs=1) as cp, \
             tc.tile_pool(name="state", bufs=1) as st, \
             tc.tile_pool(name="work", bufs=2) as wk:

            # ---- load constants ----
            def cload(shape, dt, src, tag):
                t = cp.tile(shape, dt, tag=tag)
                nc.sync.dma_start(out=t, in_=src)
                return t

            wkv = cload([61, 60], F16, wkv_d, "wkv")
            wgh = cload([128, 30], F16, wgh_d, "wgh")
            id30 = cload([128, 30], F16, id30_d, "id30")
            wenh = cload([128, 120], F16, wenh_d, "wenh")
            whh = cload([128, 120], F16, whh_d, "whh")
            wys = cload([52, 12, 120], F16, wys_d, "wys")
            wout1 = cload([1, 120], F16, wout1_d, "wout1")
            wf1 = cload([128, 30], F16, wf1_d, "wf1")
            wf2 = cload([128, 15], F16, wf2_d, "wf2")
            wf3 = cload([128, 1], F16, wf3_d, "wf3")
            b4 = cload([128, 4], F32, b4_d, "b4")
            b1s = cload([128, 12], F32, b1s_d, "b1s")
            b2 = cload([128, 1], F32, b2_d, "b2")
            bf3 = cload([1, 1], F32, bf3_d, "bf3")
            yt = cload([52, BC], F16, yt_d, "yt")

            ident = cp.tile([128, 128], F32, tag="ident")
            make_identity(nc, ident)

            # ---- phase A: acc = sum_l enc[l] via accumulating DMAs ----
            NACC = 4
            accs = [st.tile([128, 960], F32, tag=f"acc{k}", name=f"acc{k}")
                    for k in range(NACC)]
            enc_flat = [enc_d[l].rearrange("(p x) f -> p (x f)", p=128)
                        for l in range(ENC_LEN)]
            for l in range(ENC_LEN):
                if l < NACC:
                    nc.gpsimd.dma_start(out=accs[l], in_=enc_flat[l])
                else:
                    nc.gpsimd.dma_start(out=accs[l % NACC], in_=enc_flat[l],
                                        accum_op=OP.add)
            nc.vector.tensor_add(accs[0], accs[0], accs[1])
            nc.vector.tensor_add(accs[2], accs[2], accs[3])
            nc.vector.tensor_add(accs[0], accs[0], accs[2])
            acc = accs[0]

            # ---- phase A: V4 / Vg4 via PE transpose + fused [61,60] map ----
            V4 = st.tile([128, 512], F16, tag="V4")
            Vg4 = st.tile([128, 512], F16, tag="Vg4")
            with tc.tile_pool(name="psA", bufs=2, space="PSUM") as psA, \
                 tc.tile_pool(name="psV", bufs=1, space="PSUM") as psV:
                vps = psV.tile([128, 512], F32, tag="vps")
                vgps = psV.tile([128, 512], F32, tag="vgps")
                for c in range(NB):
                    et = wk.tile([61, 512], F16, tag="et")
                    nc.vector.memset(et, 1.0)
                    for jp in range(4):
                        j = 4 * c + jp
                        ptr = psA.tile([60, 128], F32, tag="ptr")
                        nc.tensor.transpose(ptr, acc[:, 60 * j:60 * j + 60], ident)
                        nc.vector.tensor_copy(et[0:60, 128 * jp:128 * jp + 128], ptr)
                    nc.tensor.matmul(vps[32 * c:32 * c + 30, :], wkv[:, 0:30], et,
                                     start=True, stop=True, tile_position=(0, 32 * c))
                    nc.tensor.matmul(vgps[32 * c:32 * c + 30, :], wkv[:, 30:60], et,
                                     start=True, stop=True, tile_position=(0, 32 * c))
                nc.vector.tensor_copy(V4, vps)
                nc.scalar.copy(Vg4, vgps)

            # ---- recurrence state ----
            h4 = st.tile([128, 512], F16, tag="h4")
            nc.sync.dma_start(out=h4, in_=h0_d)
            c4 = st.tile([128, 512], F32, tag="c4")
            nc.vector.memset(c4, 0.0)
            zeros = st.tile([128, 512], F16, tag="zeros")
            nc.vector.memset(zeros, 0.0)
            out_all = st.tile([1, 12 * BC], F16, tag="out_all")

            with tc.tile_pool(name="ps", bufs=1, space="PSUM") as ps:
                for s in range(12):
                    i = s + 1
                    # gate = sigmoid(Wg_h @ h + Vg)
                    gps = ps.tile([128, 512], F32, tag="gate")
                    for c in range(NB):
                        bs = slice(32 * c, 32 * c + 30)
                        tp = (32 * c, 32 * c)
                        nc.tensor.matmul(gps[bs, :], wgh[bs, :], h4[bs, :],
                                         start=True, stop=False, tile_position=tp)
                        nc.tensor.matmul(gps[bs, :], id30[bs, :], Vg4[bs, :],
                                         start=False, stop=True, tile_position=tp)
                    gate4 = wk.tile([128, 512], F16, tag="gate4")
                    nc.scalar.activation(out=gate4, in_=gps, func=AF.Sigmoid)
                    # enh = V + gate * (h - V)
                    hmV = wk.tile([128, 512], F16, tag="hmV")
                    nc.vector.tensor_sub(hmV, h4, V4)
                    gh = wk.tile([128, 512], F16, tag="gh")
                    nc.vector.tensor_mul(gh, gate4, hmV)
                    enh4 = wk.tile([128, 512], F16, tag="enh4")
                    nc.vector.tensor_add(enh4, gh, V4)
                    # LSTM gate pre-activations
                    ifo = ps.tile([128, 1536], F32, tag="ifo")
                    gg = ps.tile([128, 512], F32, tag="gg")
                    for b in range(4):
                        cols = slice(30 * b, 30 * b + 30)
                        dst = gg if b == 3 else ifo[:, 512 * b:512 * b + 512]
                        for c in range(NB):
                            bs = slice(32 * c, 32 * c + 30)
                            fs = slice(512 * c, 512 * c + 512)
                            tp = (32 * c, 32 * c)
                            tp0 = (0, 32 * c)
                            o = dst[bs, :]
                            nc.tensor.matmul(o, wenh[bs, cols], enh4[bs, :],
                                             start=True, stop=False, tile_position=tp)
                            nc.tensor.matmul(o, whh[bs, cols], h4[bs, :],
                                             start=False, stop=False, tile_position=tp)
                            last = (i == 1)
                            nc.tensor.matmul(o, wys[:, s, cols], yt[:, fs],
                                             start=False, stop=last, tile_position=tp0)
                            if i >= 2:
                                po = out_all[0:1, (i - 2) * BC + 512 * c:
                                             (i - 2) * BC + 512 * c + 512]
                                nc.tensor.matmul(o, wout1[:, cols], po,
                                                 start=False, stop=True,
                                                 tile_position=tp0)
                    si = wk.tile([128, 512], F16, tag="si")
                    sf = wk.tile([128, 512], F16, tag="sf")
                    so = wk.tile([128, 512], F16, tag="so")
                    tg = wk.tile([128, 512], F16, tag="tg")
                    nc.scalar.activation(out=si, in_=ifo[:, 0:512], func=AF.Sigmoid,
                                         bias=b4[:, 0:1])
                    nc.scalar.activation(out=sf, in_=ifo[:, 512:1024], func=AF.Sigmoid,
                                         bias=b4[:, 1:2])
                    nc.scalar.activation(out=so, in_=ifo[:, 1024:1536], func=AF.Sigmoid,
                                         bias=b4[:, 2:3])
                    nc.scalar.activation(out=tg, in_=gg, func=AF.Tanh,
                                         bias=b4[:, 3:4])
                    ca = wk.tile([128, 512], F32, tag="ca")
                    nc.vector.tensor_mul(ca, sf, c4)
                    cb = wk.tile([128, 512], F32, tag="cb")
                    nc.vector.tensor_mul(cb, si, tg)
                    nc.vector.tensor_add(c4, ca, cb)
                    tc4 = wk.tile([128, 512], F16, tag="tc4")
                    nc.scalar.activation(out=tc4, in_=c4, func=AF.Tanh)
                    nc.vector.tensor_mul(h4, so, tc4)
                    # output head
                    h1p = ps.tile([128, 512], F32, tag="h1")
                    for c in range(NB):
                        bs = slice(32 * c, 32 * c + 30)
                        nc.tensor.matmul(h1p[bs, :], wf1[bs, :], h4[bs, :],
                                         start=True, stop=True,
                                         tile_position=(32 * c, 32 * c))
                    z1 = wk.tile([128, 512], F16, tag="z1")
                    nc.vector.scalar_tensor_tensor(out=z1, in0=h1p,
                                                   scalar=b1s[:, s:s + 1], in1=zeros,
                                                   op0=OP.add, op1=OP.max)
                    h2p = ps.tile([128, 512], F32, tag="h2")
                    for c in range(NB):
                        bs = slice(32 * c, 32 * c + 30)
                        ks = slice(32 * c, 32 * c + 15)
                        nc.tensor.matmul(h2p[ks, :], wf2[bs, :], z1[bs, :],
                                         start=True, stop=True,
                                         tile_position=(32 * c, 32 * c))
                    z2 = wk.tile([128, 512], F16, tag="z2")
                    nc.vector.scalar_tensor_tensor(out=z2, in0=h2p, scalar=b2,
                                                   in1=zeros, op0=OP.add, op1=OP.max)
                    for c in range(NB):
                        h3p = ps.tile([1, 512], F32, tag="h3")
                        ks = slice(32 * c, 32 * c + 15)
                        nc.tensor.matmul(h3p, wf3[ks, :], z2[ks, :],
                                         start=True, stop=True,
                                         tile_position=(32 * c, 0))
                        dst = out_all[0:1, s * BC + 512 * c: s * BC + 512 * c + 512]
                        if c % 2 == 0:
                            nc.scalar.activation(out=dst, in_=h3p, func=AF.Identity,
                                                 bias=bf3)
                        else:
                            nc.vector.scalar_tensor_tensor(
                                out=dst, in0=h3p, scalar=bf3, in1=zeros[0:1, :],
                                op0=OP.add, op1=OP.add)

            # ---- output ----
            nc.gpsimd.dma_start(out=out_d.rearrange("a b -> (a b)").unsqueeze(0),
                                in_=out_all)

    nc.compile()
    return nc


def _make_weights(inp):
    W = {}
    Wv, bv = inp['Wv'], inp['bv']
    Wg, bg = inp['Wg'], inp['bg']
    Wgc = Wg[:, DEC:]
    wkv61 = np.zeros((61, 60), np.float32)
    wkv61[:60, 0:30] = (Wv / 48.0).T
    wkv61[:60, 30:60] = ((Wgc @ Wv) / 48.0).T
    wkv61[60, 0:30] = bv
    wkv61[60, 30:60] = Wgc @ bv + bg
    W['wkv61'] = wkv61.astype(np.float16)

    def banded(mat_T, M):
        out = np.zeros((128, M), np.float32)
        for c in range(NB):
            out[32 * c:32 * c + 30, :] = mat_T
        return out

    W['wgh'] = banded(Wg[:, :DEC].T, 30).astype(np.float16)
    W['id30'] = banded(np.eye(30, dtype=np.float32), 30).astype(np.float16)

    W_ih, W_hh = inp['W_ih'], inp['W_hh']
    w_enh = np.zeros((30, 120), np.float32)
    w_hh = np.zeros((30, 120), np.float32)
    for b in range(4):
        rows = slice(ROW_MAP[b], ROW_MAP[b] + 30)
        w_enh[:, 30 * b:30 * b + 30] = W_ih[rows, 4:34].T
        w_hh[:, 30 * b:30 * b + 30] = W_hh[rows, :].T
    W['w_enh'] = banded(w_enh, 120).astype(np.float16)
    W['w_hh'] = banded(w_hh, 120).astype(np.float16)

    wys = np.zeros((52, 12, 120), np.float32)
    for b in range(4):
        rows = slice(ROW_MAP[b], ROW_MAP[b] + 30)
        cols = slice(30 * b, 30 * b + 30)
        for d in range(4):
            wys[d, 0, cols] = W_ih[rows, d]
        for i in range(2, 13):
            base = (i - 1) * 4
            for d in range(1, 4):
                wys[base + d, i - 1, cols] = W_ih[rows, d - 1]
    W['wys'] = wys.astype(np.float16)
    wout1 = np.zeros((1, 120), np.float32)
    for b in range(4):
        wout1[0, 30 * b:30 * b + 30] = W_ih[ROW_MAP[b]:ROW_MAP[b] + 30, 3]
    W['wout1'] = wout1.astype(np.float16)

    bsum = inp['b_ih'] + inp['b_hh']
    b4 = np.zeros((128, 4), np.float32)
    for c in range(NB):
        for b in range(4):
            b4[32 * c:32 * c + 30, b] = bsum[ROW_MAP[b]:ROW_MAP[b] + 30]
    W['b4'] = b4

    Wf1, bf1 = inp['Wf1'], inp['bf1']
    W['wf1'] = banded(Wf1[:, :30].T, 30).astype(np.float16)
    b1s = np.zeros((128, 12), np.float32)
    for c in range(NB):
        for s in range(12):
            b1s[32 * c:32 * c + 30, s] = bf1 + Wf1[:, 30] * ((s + 1) / 12.0)
    W['b1s'] = b1s

    wf2 = np.zeros((128, 15), np.float32)
    b2 = np.zeros((128, 1), np.float32)
    for c in range(NB):
        wf2[32 * c:32 * c + 30, :] = inp['Wf2'].T
        b2[32 * c:32 * c + 15, 0] = inp['bf2']
    W['wf2'] = wf2.astype(np.float16)
    W['b2'] = b2

    wf3 = np.zeros((128, 1), np.float32)
    for c in range(NB):
        wf3[32 * c:32 * c + 15, 0] = inp['Wf3'][0, :]
    W['wf3'] = wf3.astype(np.float16)
    W['bf3'] = np.array([[float(np.asarray(inp['bf3']).reshape(-1)[0])]], np.float32)
    return W


def kernel(**inputs):
    import sys
    if '/opt/trn_rl_repo' not in sys.path:
        sys.path.insert(0, '/opt/trn_rl_repo')
    from concourse.bass_utils import run_bass_kernel_spmd

    global _PROG
    if _PROG is None:
        _PROG = _build_program()
    nc = _PROG

    inputs = {k: np.asarray(v) for k, v in inputs.items()}
    W = _make_weights(inputs)
    lod = _local_of_dev()

    enc_full = np.asarray(inputs['encoder_outputs'], np.float32)
    y_full = np.asarray(inputs['y'], np.float32)
    hid_full = np.asarray(inputs['hidden'], np.float32)

    in_maps = []
    for core in range(NCORES):
        gsl = slice(core * BC, (core + 1) * BC)
        h_dev = hid_full[gsl][lod]                       # [2048, 30]
        h0 = np.zeros((128, 512), np.float16)
        for c in range(NB):
            h0[32 * c:32 * c + 30, :] = h_dev[512 * c:512 * (c + 1)].T
        yt = y_full[gsl][lod].reshape(BC, 52).T.astype(np.float16)
        m = dict(W)
        m['enc'] = np.ascontiguousarray(enc_full[:, gsl, :])
        m['yt'] = np.ascontiguousarray(yt)
        m['h0'] = h0
        in_maps.append(m)

    res = run_bass_kernel_spmd(nc, in_maps, list(range(NCORES)))

    out = np.zeros((12, B), np.float32)
    for core in range(NCORES):
        oc = res.results[core]['out']                    # [12, 2048] dev order
        out[:, core * BC + lod] = oc
    return out


if __name__ == '__main__':
    rng = np.random.default_rng(0)
    pass
